# revision 41
# baseline (speedup 1.0000x reference)
"""Cross-attention Trainium2 kernel (8 NeuronCores, data-parallel).

Problem: B=4, C=64, H=64, W=64.
  q = conv1x1(v1, wq, bq); k = conv1x1(v2, wk, bk); v = conv1x1(v2, wv, bv)
  tokens n = (c, h) pairs (N = C*H = 4096), feature dim = W = 64
  out = softmax(q @ k^T) @ v

Sharding: core i handles batch b = i//2 and the q-token half h in
[32*(i%2), 32*(i%2+1)).  Every core needs the full v2[b] (k/v side) but only
its h-slice of v1[b] (q side).  No collectives.

Per-core algorithm:
  - scores computed TRANSPOSED: sT[j, i] = k_j . q_i with k-tokens j on
    partitions; after exp the tile is exactly the layout the P@V matmul
    streams (no attention-matrix transpose ever).
  - qT/kT held in FP16: a 32-bit moving operand streams at half rate
    through the PE, so fp32r scores matmuls cost 2x.  fp16 keeps 11
    mantissa bits (score error ~0.006 abs, irrelevant vs the bf16 P) and
    streams full rate.
  - q/k projections computed DIRECTLY in feature-major layout: x is DMA'd
    h-interleaved across the partition halves (h even -> partitions 0-63,
    odd -> 64-127); row-group-packed rank-64 matmuls with an [c, (2h, w)]
    x-slice as the stationary operand produce 256 tokens per matmul pair
    straight into PSUM (concurrent row-group matmuls MUST drain into
    different PSUM banks - same-bank is a fatal HW error).  This replaces
    the channel-major projection + 96 tiny PE transposes of the naive
    setup.  Biases (which ride the TOKEN index, token=(channel,h)) are
    applied afterwards as three whole-tensor broadcast adds.
  - a warm-up burst of dummy fp32 matmuls runs during the input DMAs so
    the HAM clock gate un-throttles the PE (1.2 -> 2.4 GHz) before the
    real compute starts, and the setup is kept dense so it stays warm.
  - no max subtraction (|s| <= ~60 here; exp fits fp32); softmax
    denominator via a ones-column appended to V.
  - main loop SOFTWARE-PIPELINED with lookahead 2; exp alternates between
    ScalarE (LUT exp) and VectorE (one-instruction Schraudolph bit-trick:
    int16(round(s*128*log2e + magic)) bitcast to bf16, ~3% per-element,
    mostly cancelled by softmax normalization; end-to-end ~5e-3).
  - V projection chunks are interleaved into pass 0's idle PE slots,
    borrowing scores PSUM tiles.
"""

import numpy as np

B, C, H, W = 4, 64, 64, 64
HH = H // 2            # h-rows per core (q-token half)
NQ = C * HH            # q tokens per core = 2048
NK = C * H             # k tokens = 4096
JB = NK // 128         # 32 j-blocks of 128 k-tokens
NP = JB // 2           # 16 row-packed j-block pairs
IP = 512               # i-span per pass (4 passes)
NCORES = 8

LOG2E = 1.4426950408889634
SCH_SCALE = 128.0 * LOG2E
SCH_BIAS = 16256.0 - 7.0   # centered so the sawtooth ratio has mean ~1
N_WARM = 22                # HAM warm-up matmuls

_CACHE = {}


def _build_nc():
    from contextlib import ExitStack

    import concourse.bass as bass
    import concourse.tile as tile
    from concourse import bacc, mybir
    from concourse.bass import ts
    from concourse.masks import make_identity

    F32 = mybir.dt.float32
    F32R = mybir.dt.float32r
    F16 = mybir.dt.float16
    BF16 = mybir.dt.bfloat16
    I16 = mybir.dt.int16
    AF = mybir.ActivationFunctionType
    ALU = mybir.AluOpType

    nc = bacc.Bacc(trn_type="TRN2", target_bir_lowering=False)

    x1_d = nc.declare_dram_parameter("x1", [C, HH, W], F32, False)
    x2_d = nc.declare_dram_parameter("x2", [C, H, W], F32, False)
    wq_d = nc.declare_dram_parameter("wq", [C, C], F32, False)
    wk_d = nc.declare_dram_parameter("wk", [C, C], F32, False)
    wv_d = nc.declare_dram_parameter("wv", [C, C], F32, False)
    bv_d = nc.declare_dram_parameter("bv", [1, C], F32, False)
    brdq_d = nc.declare_dram_parameter("brdq", [128, 512], mybir.dt.uint16, False)
    brdk_d = nc.declare_dram_parameter("brdk", [128, 512], mybir.dt.uint16, False)
    out_d = nc.declare_dram_parameter("out", [C, HH, W], F32, True)

    with ExitStack() as ctx:
        tc = ctx.enter_context(tile.TileContext(nc))
        cp = ctx.enter_context(tc.tile_pool(name="const", bufs=1))

        ident = cp.tile([128, 128], F32)
        make_identity(nc, ident[:, :])

        # prewarm the exp table set while input DMAs run
        warm = cp.tile([128, 2], F32)
        nc.vector.memset(warm[:, :], 0.0)
        nc.scalar.activation(warm[:, 0:1], warm[:, 1:2], AF.Exp)

        # h-interleaved x copies: h even -> partitions 0-63, odd -> 64-127
        x1_pk = cp.tile([128, HH // 2, W], F32R)
        x2_pk = cp.tile([128, H // 2, W], F32R)
        # channel-major x2 for the V projection (+ ones row for bias):
        # fp16 so the V matmuls stream full-rate; DMA to fp32 staging then
        # cast (DMA cannot convert)
        x2_st = cp.tile([C, H * W], F32)
        x2_sb = cp.tile([C + 1, H * W], F16)
        nc.gpsimd.memset(x2_sb[C : C + 1, :], 1.0)

        # vf_aug (128, JB, 65) bf16: partition p of block jb = v-token
        # (h = 2*jb + p//64, o = p%64); col 64 = 1.0 (denominator trick).
        vf = cp.tile([128, JB, 65], BF16)
        nc.gpsimd.memset(vf[:, :, :], 1.0)

        # brd_b[qk]: bias[o] tiled along the whole token axis, identical
        # on all w-partitions (host-precomputed fp16, doubled on-chip)
        brd_bq = cp.tile([128, NQ], F16)
        brd_bk = cp.tile([128, NQ], F16)

        # DMA queue order = criticality: x1 (Q path), weights/biases,
        # x2 h-interleaved (K path), x2 channel-major (V path, needed
        # deepest into pass 0)
        for h2 in range(2):
            nc.sync.dma_start(
                x1_pk[ts(h2, C), :, :],
                x1_d[:, :, :].rearrange("c (hh two) w -> c hh two w", two=2)[
                    :, :, h2, :
                ].bitcast(F32R),
            )
        w_sb = {}
        for name, wd in (("q", wq_d), ("k", wk_d), ("v", wv_d)):
            t = cp.tile([C, C], F32, tag=f"w_{name}")
            nc.sync.dma_start(t[:, :], wd[:, :])
            w_sb[name] = t
        wv_st = cp.tile([C + 1, C], F32, tag="wv_st")
        nc.sync.dma_start(wv_st[C : C + 1, :], bv_d[:, :])
        nc.sync.dma_start(brd_bq[:, 0:512], brdq_d[:, :].bitcast(F16))
        nc.sync.dma_start(brd_bk[:, 0:512], brdk_d[:, :].bitcast(F16))
        for h2 in range(2):
            nc.sync.dma_start(
                x2_pk[ts(h2, C), :, :],
                x2_d[:, :, :].rearrange("c (hh two) w -> c hh two w", two=2)[
                    :, :, h2, :
                ].bitcast(F32R),
            )
        for ch in range(2):
            nc.sync.dma_start(
                x2_st[:, ts(ch, H * W // 2)],
                x2_d[:, :, :].rearrange("c h w -> c (h w)")[
                    :, ts(ch, H * W // 2)
                ],
            )

        # wqT2/wkT2: [c, o] on both partition halves (rhs of the direct
        # projections); wTv: [c, o] + bias row (lhsT of the V projection)
        wqT2 = cp.tile([128, C], F32R)
        wkT2 = cp.tile([128, C], F32R)
        wTv = cp.tile([C + 1, C], F16)

        with tc.tile_pool(name="pp0", bufs=2, space="PSUM") as pp0:
            # HAM warm-up: dummy fp32 matmuls (quarter-rate => long busy
            # per instruction) while the DMAs stream in
            wps = pp0.tile([128, 128], F32, tag="warmmm")
            for _ in range(N_WARM):
                nc.tensor.matmul(wps[:, :], lhsT=ident[:, :], rhs=ident[:, :],
                                 start=True, stop=True)

            for name, dst in (("q", wqT2), ("k", wkT2)):
                ps = pp0.tile([C, C], F32, tag="wT_ps")
                nc.tensor.transpose(ps[:, :], w_sb[name][:, :], ident[0:C, 0:C])
                nc.vector.tensor_copy(dst[0:C, :], ps[:, :])
                nc.vector.tensor_copy(dst[C : 2 * C, :], ps[:, :])
            ps = pp0.tile([C, C], F32, tag="wT_ps")
            nc.tensor.transpose(ps[:, :], w_sb["v"][:, :], ident[0:C, 0:C])
            nc.vector.tensor_copy(wv_st[0:C, :], ps[:, :])
            nc.vector.tensor_copy(wTv[:, :], wv_st[:, :])
            for brd in (brd_bq, brd_bk):
                nc.vector.tensor_copy(brd[:, 512:1024], brd[:, 0:512])
                nc.vector.tensor_copy(brd[:, 1024:2048], brd[:, 0:1024])


        # ---- direct feature-major q/k projections (fp16 outputs) ----
        # qT2: (w, i=h*64+o) duplicated on both partition halves
        # kT2: (w, j) even j-blocks on partitions 0-63, odd on 64-127
        qT2 = cp.tile([128, NQ], F16)
        kT2 = cp.tile([128, NK // 2], F16)

        with tc.tile_pool(name="ppqk", bufs=2, space="PSUM") as ppqk:
            def qk_group(g, x_pk, wT2, is_q):
                # one group = 16 h's (tokens [1024g, 1024(g+1))).  The
                # stationary operand covers TWO adjacent h-pairs:
                # lhsT [c, (hh2, w)] -> psum partitions (hh2, w).  Eight
                # matmuls per group; h-parity hp lands in separate PSUM
                # banks: ps[64*hh2 + w, hp*512 + uu*64 + o]
                ps = ppqk.tile([128, 1024], F32, tag="qk")
                for uu in range(4):
                    hh0 = 8 * g + 2 * uu
                    for hp in range(2):
                        nc.tensor.matmul(
                            ps[:, hp * 512 + uu * C :][:, 0:C],
                            lhsT=x_pk[ts(hp, C), hh0 : hh0 + 2, :],
                            rhs=wT2[ts(hp, C), :],
                            start=True, stop=True,
                        )
                # psum (64*hh2 + w, hp*512 + uu*64 + o) ->
                #   h = 2*(8g + 2uu + hh2) + hp
                for hh2 in range(2):
                    src = ps[ts(hh2, C), :].rearrange(
                        "p (hp uu o) -> p uu hp o", hp=2, o=C
                    )
                    if is_q:
                        # token-in-group X = 4uu + 2hh2 + hp
                        dst = qT2[0:C, ts(g, 1024)].rearrange(
                            "p (uu hh2x hp o) -> p hh2x uu hp o",
                            uu=4, hh2x=2, hp=2,
                        )[:, hh2, :, :, :]
                    else:
                        # j-block jb = 8g + 2uu + hh2: parity hh2,
                        # pair p = 4g + uu
                        dst = kT2[64 * hh2 : 64 * hh2 + C, ts(g, 512)].rearrange(
                            "p (uu hp o) -> p uu hp o", uu=4, hp=2
                        )
                    eng = nc.vector if (hh2 == 0) else nc.scalar
                    if eng is nc.vector:
                        nc.vector.tensor_copy(dst, src[:, 0:4, :, :])
                    else:
                        nc.scalar.copy(dst, src[:, 0:4, :, :])

            # Q: one 32-h group; i-token order (hh2, hp, uu, o) chosen so
            # the PSUM->SBUF copy per hh2-half is a single 2D fused
            # bias-add (h = 4*uu + 2*hh2 + hp)
            psq = ppqk.tile([128, 1024], F32, tag="qk")
            for uu in range(8):
                for hp in range(2):
                    nc.tensor.matmul(
                        psq[:, hp * 512 + uu * C :][:, 0:C],
                        lhsT=x1_pk[ts(hp, C), 2 * uu : 2 * uu + 2, :],
                        rhs=wqT2[ts(hp, C), :],
                        start=True, stop=True,
                    )
            for hh2 in range(2):
                nc.vector.scalar_tensor_tensor(
                    qT2[0:C, ts(hh2, 1024)], psq[ts(hh2, C), :], 1.0,
                    brd_bq[0:C, 0:1024], ALU.mult, ALU.add,
                )
            nc.vector.tensor_copy(qT2[C : 2 * C, :], qT2[0:C, :])
            for g in range(NK // 1024):
                qk_group(g, x2_pk, wkT2, False)

            # biases ride the token index: one broadcast add per tensor
            # half (fp16 in-place)
            nc.vector.scalar_tensor_tensor(
                kT2[0:C, :], kT2[0:C, :], 1.0, brd_bk[0:C, :], ALU.mult, ALU.add
            )
            nc.vector.scalar_tensor_tensor(
                kT2[C : 2 * C, :], kT2[C : 2 * C, :], 1.0, brd_bk[C : 2 * C, :],
                ALU.mult, ALU.add,
            )

        # ---- main attention loop: 4 passes over i, row-packed j pairs ----
        LOOKAHEAD = 2
        outT_sb = cp.tile([C + 1, NQ], F32)
        with (
            tc.tile_pool(name="outp", bufs=1, space="PSUM") as op_pool,
            tc.tile_pool(name="sp", bufs=LOOKAHEAD + 1, space="PSUM") as sp,
            tc.tile_pool(name="ppool", bufs=4) as p_pool,
            tc.tile_pool(name="tp2", bufs=1, space="PSUM") as tp2,
            tc.tile_pool(name="opool", bufs=4) as o_pool,
            tc.tile_pool(name="rpool", bufs=4) as r_pool,
        ):
            outT_ps = None
            sps_ring = {}
            pt_ring = {}

            def emit_scores(ih, p):
                i0 = ih * IP
                sps = sp.tile([128, 2 * IP], F32, tag="scores")
                for blk in range(2):
                    half = 64 * blk
                    nc.tensor.matmul(
                        sps[:, ts(blk, IP)],
                        lhsT=kT2[half : half + 64, ts(p, 128)],
                        rhs=qT2[half : half + 64, i0 : i0 + IP],
                        start=True, stop=True,
                    )
                sps_ring[(ih, p)] = sps

            def emit_exp(ih, p):
                sps = sps_ring.pop((ih, p))
                pt = p_pool.tile([128, 2 * IP], BF16, tag="p")
                if p % 2 == 0:
                    nc.scalar.activation(pt[:, :], sps[:, :], AF.Exp)
                else:
                    # Schraudolph bit-trick exp on the DVE
                    nc.vector.tensor_scalar(
                        pt[:, :].bitcast(I16), sps[:, :], SCH_SCALE, SCH_BIAS,
                        ALU.mult, ALU.add,
                    )
                pt_ring[(ih, p)] = pt

            def emit_pv(ih, p):
                pt = pt_ring.pop((ih, p))
                for blk in range(2):
                    jb = 2 * p + blk
                    nc.tensor.matmul(
                        outT_ps[:, :],
                        lhsT=vf[:, jb, :],
                        rhs=pt[:, ts(blk, IP)],
                        start=(p == 0 and blk == 0),
                        stop=(p == NP - 1 and blk == 1),
                    )

            def emit_x2cast(ch):
                nc.vector.tensor_copy(
                    x2_sb[0:C, ts(ch, 1024)], x2_st[:, ts(ch, 1024)]
                )

            def emit_projv(ch):
                # V chunk ch (16 h's): borrows a scores PSUM tile; fills
                # vf blocks [8ch, 8ch+8)
                ps = sp.tile([128, 2 * IP], F32, tag="scores")
                for c2 in range(2):
                    nc.tensor.matmul(
                        ps[0:C, ts(c2, 512)],
                        lhsT=wTv[:, :],
                        rhs=x2_sb[:, ch * 1024 + c2 * 512 :][:, 0:512],
                        start=True, stop=True,
                    )
                pv = ps[0:C, :].rearrange("p (h2 h1 w) -> p h1 h2 w", h1=2, w=W)
                for h1 in range(2):
                    dst = vf[64 * h1 : 64 * (h1 + 1), ts(ch, 8), 0:W]
                    if h1 == 0:
                        nc.scalar.copy(dst, pv[:, h1, :, :])
                    else:
                        nc.vector.tensor_copy(dst, pv[:, h1, :, :])

            def emit_drain_head(ih, acc_ps):
                # copy pass ih's accumulator (with its denominator row) to
                # SBUF; per-tile normalization happens post-transpose where
                # the denominator is a per-partition scalar
                i0 = ih * IP
                nc.vector.tensor_copy(outT_sb[:, i0 : i0 + IP], acc_ps[:, :])

            def emit_drain_tile(ih, tt, pool=None):
                t = ih * (IP // 128) + tt
                ps = (pool or tp2).tile(
                    [128, C + 1], F32,
                    tag="ot" if pool is None else "scores",
                )
                nc.tensor.transpose(
                    ps[:, :], outT_sb[:, ts(t, 128)], ident[0 : C + 1, 0 : C + 1]
                )
                rec = r_pool.tile([128, 1], F32, tag="rec")
                nc.vector.reciprocal(rec[:, :], ps[:, C : C + 1])
                ot = o_pool.tile([128, C], F32, tag="o")
                nc.vector.tensor_scalar_mul(ot[:, :], ps[:, 0:C], rec[:, 0:1])
                # i = hh2*1024 + hp*512 + uu*64 + o; rows p = up*64 + o ->
                # out[o, h = 4*(2*tt+up) + ih, :]
                dest = out_d[:, :, :].rearrange(
                    "o (hb r) w -> o hb r w", r=4
                )[:, 2 * tt : 2 * tt + 2, ih, :].rearrange("o hb w -> hb o w")
                nc.sync.dma_start(dest, ot[:, :])

            NPASS = NQ // IP
            emit_x2cast(0)
            for ih in range(NPASS):
                prev_outT_ps = outT_ps
                emit_scores(ih, 0)
                if ih > 0:
                    emit_drain_head(ih - 1, prev_outT_ps)
                for p in range(1, LOOKAHEAD):
                    emit_scores(ih, p)
                outT_ps = op_pool.tile([C + 1, IP], F32, tag="outT")
                for p in range(NP):
                    if ih == 0:
                        if p in (0, 3, 7):
                            emit_x2cast(p // 3 + 1)
                        if p in (0, 1, 5, 9):
                            emit_projv((p + 3) // 4)
                    if ih > 0 and p in (2, 5, 8, 11):
                        emit_drain_tile(ih - 1, (p - 2) // 3)
                    emit_exp(ih, p)
                    # next pair's scores emitted BETWEEN the two P@V
                    # accumulating matmuls: the independent stream hides
                    # the same-bank PSUM accumulation turnaround
                    pt = pt_ring.pop((ih, p))
                    nc.tensor.matmul(
                        outT_ps[:, :], lhsT=vf[:, 2 * p, :],
                        rhs=pt[:, ts(0, IP)], start=(p == 0), stop=False,
                    )
                    if p + LOOKAHEAD < NP:
                        emit_scores(ih, p + LOOKAHEAD)
                    nc.tensor.matmul(
                        outT_ps[:, :], lhsT=vf[:, 2 * p + 1, :],
                        rhs=pt[:, ts(1, IP)], start=False, stop=(p == NP - 1),
                    )
            emit_drain_head(NPASS - 1, outT_ps)
            for tt in range(IP // 128):
                # the scores pool is idle by now: borrow its buffers so the
                # final four transpose->normalize->DMA chains pipeline
                emit_drain_tile(NPASS - 1, tt, pool=sp)

    nc.compile()
    return nc


def _get_nc():
    if "nc" not in _CACHE:
        _CACHE["nc"] = _build_nc()
    return _CACHE["nc"]


def _in_maps(v1, v2, wq, bq, wk, bk, wv, bv):
    brdq = np.tile(
        np.asarray(bq, np.float32).astype(np.float16).view(np.uint16).reshape(1, C),
        (128, 8),
    )
    brdk = np.tile(
        np.asarray(bk, np.float32).astype(np.float16).view(np.uint16).reshape(1, C),
        (128, 8),
    )
    maps = []
    for core in range(NCORES):
        b, half = divmod(core, 2)
        maps.append({
            "x1": np.ascontiguousarray(
                v1[b, :, half * HH : (half + 1) * HH, :], dtype=np.float32
            ),
            "x2": np.ascontiguousarray(v2[b], dtype=np.float32),
            "wq": np.ascontiguousarray(wq, dtype=np.float32),
            "wk": np.ascontiguousarray(wk, dtype=np.float32),
            "wv": np.ascontiguousarray(wv, dtype=np.float32),
            "bv": np.ascontiguousarray(bv, dtype=np.float32).reshape(1, C),
            "brdq": brdq,
            "brdk": brdk,
        })
    return maps


def _gather(results, v1):
    out = np.zeros((B, C, H, W), dtype=np.float32)
    for core in range(NCORES):
        b, half = divmod(core, 2)
        out[b, :, half * HH : (half + 1) * HH, :] = results[core]["out"]
    return out


def _run(trace=False, **inputs):
    from concourse.bass_utils import run_bass_kernel_spmd

    nc = _get_nc()
    maps = _in_maps(**inputs)
    res = run_bass_kernel_spmd(
        nc, maps, core_ids=list(range(NCORES)), trace=trace
    )
    return _gather(res.results, inputs["v1"]), res


def kernel(**inputs):
    out, _ = _run(trace=False, **inputs)
    return out
